# revision 50
# baseline (speedup 1.0000x reference)
"""Trainium2 Bass kernel for nn_Head_72507637891886.

Computes r = exp(-(|k|_F^2+|q|_F^2)/2) * mean(cosh((k+q) @ w), -1) where
k = x@wk+bk, q = x@wq+bq, w = sqrt(32) * w_raw.T / |w_raw|_F.

Strategy: data-parallel over batch (2 batches = 8192 tokens per core, 8 cores).
The kernel is HBM-bound on streaming x (modeled 360 GB/s, fully serialized
across DMA queues), so x is shipped to the device as bf16 ([E, TOK] transposed
on host): halves the stream from 93us to 47us. The small matmul operands
(wkq, ws8, mean weights) are fused into one [128, 521] bf16 "wall" and the
f32 biases into a [64, 2] tensor, so two small DMAs precede the x stream.
Per block (15x512 + 2x256 tokens), work split so no engine exceeds the
2.9us DMA period (ACT ~1.4us, DVE ~1.4us, PE ~2.4us):
  - 8 accumulating bf16 matmuls -> kq^T [64, blk] PSUM f32
  - DVE tensor_scalar_add(+bias) -> kqb bf16 (k,q biased, transposed)
  - ACT Square+bias with accum_out -> per-feature sum-of-squares column
  - matmul with stacked [+wS | -wS] -> [y^T; -y^T] [8, blk] PSUM
  - ACT Exp -> [e^y; e^-y] bf16, matmul with 0.125 -> mean(cosh) [1, blk]
  - DVE tensor_scalar_add(+0) -> r row
Tail: rout [1, TOK] (SP queue) and ssout [64, NBLK] (ACT queue) ship in
parallel. Host all-reduces the sum-of-squares partials and applies the
exp(-z2/2) scale (underflows to 0 for this input scale).
"""

import numpy as np

B, T, E, D = 16, 4096, 1024, 32
OMEGA = 4
NCORES = 8
TOK = B * T // NCORES  # 8192 tokens per core
KC = E // 128          # 8 contraction chunks
BLOCKS = [512] * 15 + [256, 256]
NBLK = len(BLOCKS)
WALL_F = 521           # 512 wkq | 8 ws8 | 1 mean-w

_CACHE = {}
LAST_RESULTS = None  # BassKernelResults from the most recent run (for test.py)
LAST_PROFILE = None
LAST_OUTS = None
TRACE = False


def _build_bass():
    import concourse.mybir as mybir
    import concourse.tile as tile
    from concourse import bacc

    f32 = mybir.dt.float32
    bf16 = mybir.dt.bfloat16
    AF = mybir.ActivationFunctionType

    nc = bacc.Bacc()
    xt = nc.declare_dram_parameter("xt", [E, TOK], bf16, isOutput=False)
    wall = nc.declare_dram_parameter("wall", [128, WALL_F], bf16, isOutput=False)
    bias3 = nc.declare_dram_parameter("bias3", [128, 3], f32, isOutput=False)
    rout = nc.declare_dram_parameter("rout", [1, TOK - 256], f32, isOutput=True)
    # ship = [ss_cols | rtail]: rows 0:64 cols 0:NBLK-2 hold per-block
    # sum-of-squares columns (the last two blocks' ss is computed on host
    # from kqout); all 128 rows of the last 2 cols hold the final block's
    # r values (tokens on partitions)
    ship = nc.declare_dram_parameter("ship", [128, NBLK + 2], f32, isOutput=True)
    # biased kq of the last two blocks; host sums their squares so no ACT
    # Square sits in the kernel's drain
    kqout = nc.declare_dram_parameter("kqout", [2 * D, 512], bf16, isOutput=True)

    with tile.TileContext(nc) as tc:
        with (
            tc.tile_pool(name="const", bufs=1) as const,
            tc.tile_pool(name="xp", bufs=5) as xp,
            tc.tile_pool(name="work", bufs=3) as work,
            tc.tile_pool(name="acc", bufs=1) as acc,
            tc.tile_pool(name="kqps", bufs=3, space="PSUM") as kqps,
            tc.tile_pool(name="yps", bufs=2, space="PSUM") as yps,
            tc.tile_pool(name="mps", bufs=2, space="PSUM") as mps,
            tc.tile_pool(name="ytps", bufs=1, space="PSUM") as ytps,
        ):
            # Warm up the PE pstate ramp (0.65 -> 2.4 GHz after ~3us busy)
            # with dummy matmuls on a memset tile, sized to keep PE busy
            # until block 0's data lands so its real matmuls run full speed.
            # The scratch PSUM tile borrows a kqps slot (released right away).
            wu = const.tile([128, 512], bf16)
            nc.vector.memset(wu, 0.0)
            wu_ps = kqps.tile([64, 512], f32, tag="kq", name="wu_ps")
            for _ in range(10):
                nc.tensor.matmul(wu_ps, wu[:, 0:64], wu, start=True, stop=True)
            wall_sb = const.tile([128, WALL_F], bf16)
            bias3_sb = const.tile([128, 3], f32)
            wkq_sb = wall_sb[:, 0:512]
            ws8_sb = wall_sb[0:64, 512:520]
            c8w_sb = wall_sb[0:8, 520:521]
            bkq_sb = bias3_sb[0:64, 0:1]
            zero8_sb = bias3_sb[0:8, 1:2]
            ln8_sb = bias3_sb[0:128, 2:3]   # ln(1/8): exp(y+ln8) = e^y / 8

            ship_sb = acc.tile([128, NBLK + 2], f32)
            ss_cols = ship_sb[0:64, 0:NBLK]
            rt_sb = ship_sb[:, NBLK : NBLK + 2]
            r_sb = acc.tile([1, TOK - 256], f32)
            kqship = acc.tile([2 * D, 512], bf16)

            # Software pipeline, staggered so every PE instruction's deps are
            # satisfied when PE (in-order) reaches it: block i emits its own
            # kq matmuls, y8 for block i-1, and mean for block i-2. Stage
            # state carried across iterations:
            kqb_t = [None] * NBLK   # biased kq (bf16 SBUF)
            y8_t = [None] * NBLK    # [y; -y] PSUM
            e_t = [None] * NBLK     # [e^y; e^-y] (bf16 SBUF)
            m_t = [None] * NBLK     # mean-cosh PSUM
            off = [0] * NBLK

            def stage_y8(i):
                y8_t[i] = yps.tile([2 * OMEGA, BLOCKS[i]], f32, tag="y8", name="y8")
                nc.tensor.matmul(y8_t[i], ws8_sb, kqb_t[i], start=True, stop=True)

            def stage_exp(i):
                e_t[i] = work.tile([2 * OMEGA, BLOCKS[i]], bf16, tag="e", name="e")
                nc.scalar.activation(e_t[i], y8_t[i], AF.Exp, bias=zero8_sb)

            def stage_mean(i):
                m_t[i] = mps.tile([1, BLOCKS[i]], f32, tag="m", name="m")
                nc.tensor.matmul(m_t[i], c8w_sb, e_t[i], start=True, stop=True)

            def stage_r(i):
                nc.vector.tensor_scalar_add(
                    r_sb[:, off[i] : off[i] + BLOCKS[i]], m_t[i], 0.0
                )

            t0 = 0
            for ib, blk in enumerate(BLOCKS):
                off[ib] = t0
                x_tile = xp.tile([128, KC, blk], bf16, tag="x")
                if ib == NBLK - 1:
                    # split the final transfer by contraction rows: the first
                    # half's matmuls start a transfer earlier, shortening the
                    # post-stream drain (same bytes, both runs >= 512B)
                    nc.sync.dma_start(
                        out=x_tile[:, 0:5, :],
                        in_=xt[0 : 5 * 128, t0 : t0 + blk].rearrange(
                            "(c p) t -> p c t", p=128
                        ),
                    )
                    nc.sync.dma_start(
                        out=x_tile[:, 5:KC, :],
                        in_=xt[5 * 128 : E, t0 : t0 + blk].rearrange(
                            "(c p) t -> p c t", p=128
                        ),
                    )
                else:
                    nc.sync.dma_start(
                        out=x_tile,
                        in_=xt[:, t0 : t0 + blk].rearrange("(c p) t -> p c t", p=128),
                    )
                if ib == 0:
                    # small operands load behind block 0's transfer so the
                    # x stream owns the DMA engines from t=0
                    nc.sync.dma_start(out=wall_sb, in_=wall[:])
                    nc.sync.dma_start(out=bias3_sb, in_=bias3[:])

                kq_ps = kqps.tile([2 * D, blk], f32, tag="kq")
                for c in range(KC):
                    nc.tensor.matmul(
                        kq_ps,
                        wkq_sb[:, c * 64 : (c + 1) * 64],
                        x_tile[:, c, :],
                        start=(c == 0),
                        stop=(c == KC - 1),
                    )
                if ib >= 1:
                    stage_y8(ib - 1)
                if ib >= 2:
                    stage_mean(ib - 2)

                # biased kq for the y8 matmul (DVE copy+bias); the last two
                # blocks land in kqship, which is DMAed out for host-side
                # sum-of-squares
                if ib >= NBLK - 2:
                    kqb_t[ib] = kqship[:, (ib - (NBLK - 2)) * 256 :
                                       (ib - (NBLK - 2)) * 256 + blk]
                else:
                    kqb_t[ib] = work.tile([2 * D, blk], bf16, tag="kqb",
                                          name="kqb")
                nc.vector.tensor_scalar_add(kqb_t[ib], kq_ps, bkq_sb)

                if ib >= 1:
                    stage_exp(ib - 1)
                if ib < NBLK - 2:
                    # (k+bk)^2 and (q+bq)^2 summed along tokens via accum_out;
                    # the squared tile itself is a write-only scratch
                    sq = work.tile([2 * D, blk], f32, tag="sq")
                    nc.scalar.activation(
                        sq, kq_ps, AF.Square, bias=bkq_sb,
                        accum_out=ss_cols[:, ib : ib + 1],
                    )
                if ib >= 2:
                    stage_r(ib - 2)
                t0 += blk

            # Drain. The last block takes a short token-partition path:
            # y8t = kqb^T @ ws8 puts tokens on partitions, so one Exp with
            # bias=ln(1/8) and accum_out yields 0.125*sum_j e^y per token —
            # no mean matmul or r copy on the critical path.
            L = NBLK - 1
            y8t = ytps.tile([128, 16], f32)
            nc.tensor.matmul(
                y8t[:, 0:8], kqb_t[L][:, 0:128], ws8_sb, start=True, stop=True
            )
            nc.tensor.matmul(
                y8t[:, 8:16], kqb_t[L][:, 128:256], ws8_sb, start=True, stop=True
            )
            stage_mean(L - 1)
            stage_r(L - 1)
            etA = work.tile([128, 8], bf16, tag="et", name="etA")
            nc.scalar.activation(
                etA, y8t[:, 0:8], AF.Exp, bias=ln8_sb,
                accum_out=rt_sb[:, 0:1],
            )
            etB = work.tile([128, 8], bf16, tag="et", name="etB")
            nc.scalar.activation(
                etB, y8t[:, 8:16], AF.Exp, bias=ln8_sb,
                accum_out=rt_sb[:, 1:2],
            )

            # tail outputs on separate HWDGE queues so their DGE stages
            # overlap; kqout's wait (kqb of the last block) fires earliest
            nc.sync.dma_start(out=kqout[:], in_=kqship)
            nc.sync.dma_start(out=rout[:], in_=r_sb)
            nc.scalar.dma_start(out=ship[:], in_=ship_sb)
    nc.compile()
    return nc


def _get_nc():
    if "nc" not in _CACHE:
        _CACHE["nc"] = _build_bass()
    return _CACHE["nc"]


def _make_inputs(x, wq, bq, wk, bk, w_raw):
    import ml_dtypes

    bf16 = ml_dtypes.bfloat16
    # replicated small operands, fused into one [128, WALL_F] bf16 wall
    wkq = np.concatenate([wk, wq], axis=1)  # [E, 64]
    wkq_p = wkq.reshape(KC, 128, 2 * D).transpose(1, 0, 2).reshape(128, 512)
    wt = w_raw.T.astype(np.float32)  # [D, OMEGA]
    norm = np.sqrt(np.sum(wt ** 2, dtype=np.float32))
    w = (np.float32(np.sqrt(np.float32(D))) * (wt / norm)).astype(np.float32)
    wS = np.concatenate([w, w], axis=0)  # [64, OMEGA]
    ws8 = np.concatenate([wS, -wS], axis=1)  # [64, 8]

    wall = np.zeros((128, WALL_F), dtype=np.float32)
    wall[:, 0:512] = wkq_p
    wall[0:64, 512:520] = ws8
    wall[0:8, 520] = 0.125
    wall_b = wall.astype(bf16)

    bias3 = np.zeros((128, 3), dtype=np.float32)
    bias3[0:64, 0] = np.concatenate([bk, bq])
    bias3[:, 2] = np.float32(np.log(0.125))

    in_maps = []
    bpc = B // NCORES
    for c in range(NCORES):
        xt = np.ascontiguousarray(
            x[c * bpc : (c + 1) * bpc].reshape(TOK, E).astype(bf16).T
        )  # [E, TOK] bf16
        in_maps.append({"xt": xt, "wall": wall_b, "bias3": bias3})
    return in_maps


def kernel(x, wq, bq, wk, bk, wv, bv, w_raw):
    global LAST_RESULTS, LAST_OUTS
    from concourse.bass_utils import run_bass_kernel_spmd

    x = np.asarray(x, dtype=np.float32)
    wq = np.asarray(wq, dtype=np.float32)
    bq = np.asarray(bq, dtype=np.float32)
    wk = np.asarray(wk, dtype=np.float32)
    bk = np.asarray(bk, dtype=np.float32)
    w_raw = np.asarray(w_raw, dtype=np.float32)

    in_maps = _make_inputs(x, wq, bq, wk, bk, w_raw)

    nc = _get_nc()
    res = run_bass_kernel_spmd(
        nc, in_maps, core_ids=list(range(NCORES)), trace=False
    )
    LAST_RESULTS = res
    results = res.results
    LAST_OUTS = results

    r_parts = []
    ss = 0.0
    for out in results:
        sh = out["ship"].reshape(128, NBLK + 2)
        kq2 = out["kqout"].reshape(2 * D, 512).astype(np.float32)
        r_parts.append(out["rout"].reshape(TOK - 256))
        r_parts.append(sh[:, NBLK])
        r_parts.append(sh[:, NBLK + 1])
        ss += float(sh[0:64, 0 : NBLK - 2].sum(dtype=np.float64))
        ss += float(np.sum(kq2.astype(np.float64) ** 2))

    with np.errstate(under="ignore"):
        a = np.float32(np.exp(np.float64(-ss / 2.0)))
    r = (a * np.concatenate(r_parts)).reshape(B, T).astype(np.float32)
    return r
